# revision 12
# baseline (speedup 1.0000x reference)
"""GCN autoencoder (2x GCNConv + Linear) on 8 Trainium2 NeuronCores — v2.

Sharding (per hint): nodes in contiguous chunks across 8 cores; edges
partitioned by destination node.  Per conv, each core gathers source rows
with dma_gather (4 SWDGE queues, ~3.8K-row calls — the measured sweet
spot), builds pure one-hot indicators ON DEVICE (DVE iota==dc with a
stride-0 broadcast AP, times isq[src] for conv1), and scatter-adds via PE
matmuls into per-128-dst-block PSUM tiles.  Degree normalization is
factored: isq[src] rides in the conv1 indicator / is pre-folded into the
conv2 source features; isq[dst] is applied as a per-column DVE multiply
on the PSUM read-out.  Self-loops skip the gather entirely for BOTH convs (their contribution
is added during PSUM post-processing: isq*x_fm for conv1 via a local
PE-transpose of this core's x chunk, t2'[:, d] for conv2).  The halo
exchange is two Shared-DRAM AllGathers split at node 3200 (= 5 segments)
so conv2 h=0 gathers start while conv1 still computes.
"""

import numpy as np

import concourse.bass as bass
import concourse.tile as tile
from concourse import bacc, mybir
from concourse.bass_utils import run_bass_kernel_spmd

# ---------------- problem constants (hardcoded per contract) ----------------
N = 50000
E = 500000
D_IN = 128
D_HID = 128  # conv1 out = 2*D_HID = 256
D_OUT = 6
CORES = 8
CHUNK = N // CORES       # 6250
W = 128                  # dst-block width
BPG = 5                  # blocks per group (group == transform segment)
NB = -(-CHUNK // W)      # 49 blocks
NSEG = -(-NB // BPG)     # 10 groups/segments
SLAB = 512
AG_SEG_A = (NSEG + 1) // 2 - 1   # last segment of halo piece A (4)
JSPLIT = BPG * W * (AG_SEG_A + 1)  # 3200: piece A = nodes [0, 3200) per chunk
H1TAB = N // 2           # conv1 gather table split at row 25000

F32 = mybir.dt.float32
BF16 = mybir.dt.bfloat16
I16 = mybir.dt.int16
NPBF16 = np.dtype("bfloat16")


def _cd(a, b):
    return -(-a // b)


def _wrap_idx(ix):
    """[L] int -> [128, L//16] int16 wrapped in 16 partitions, replicated x8."""
    n = len(ix)
    arr = np.zeros((16, n // 16), np.int16)
    arr[np.arange(n) % 16, np.arange(n) // 16] = ix.astype(np.int16)
    return np.tile(arr, (8, 1))


def _plan_conv(src, dst, nmv, h, idx_vals, n_cores, chunk, w, bpg):
    """Edge tiling plan, uniform across cores.

    Cells are (group, h, block); each cell's edges are packed into
    128-row tiles, padded to the max count over cores.  Returns global
    structure plus per-core idx / dc / nm streams.
    """
    nb = _cd(chunk, w)
    g_n = _cd(nb, bpg)
    m = dst // chunk
    dl = dst % chunk
    b = dl // w
    g = b // bpg
    bl = b % bpg

    cnt = np.zeros((n_cores, g_n, 2, bpg), np.int64)
    np.add.at(cnt, (m, g, h, bl), 1)
    t_cell = -(-cnt.max(axis=0) // 128)  # [G,2,BPG]
    t_tot = int(t_cell.sum())

    tile_base = np.zeros((g_n, 2, bpg), np.int64)
    run = 0
    for gg in range(g_n):
        for hh in range(2):
            for bb in range(bpg):
                tile_base[gg, hh, bb] = run
                run += t_cell[gg, hh, bb]

    t_call = t_cell.sum(axis=2)          # [G,2] tiles per gather call
    l_gh = t_call * 128
    call_base = np.zeros((g_n, 2), np.int64)
    off16 = np.zeros((g_n, 2), np.int64)
    run_t, run_i = 0, 0
    for gg in range(g_n):
        for hh in range(2):
            call_base[gg, hh] = run_t
            off16[gg, hh] = run_i
            run_t += t_call[gg, hh]
            run_i += l_gh[gg, hh] // 16
    it16 = run_i

    flat_base = tile_base.reshape(-1)
    eap = t_tot * 128
    per_core = []
    for mm in range(n_cores):
        sel = np.nonzero(m == mm)[0]
        key = (g[sel] * 2 + h[sel]) * bpg + bl[sel]
        order = np.argsort(key, kind="stable")
        sel = sel[order]
        key = key[order]
        kcnt = np.bincount(key, minlength=g_n * 2 * bpg)
        starts = np.concatenate([[0], np.cumsum(kcnt)[:-1]])
        rank = np.arange(len(sel)) - starts[key]
        pos = flat_base[key] * 128 + rank

        dc = np.full(eap, -5.0, np.float32)
        nm = np.zeros(eap, np.float32)
        ix = np.zeros(eap, np.int64)
        dc[pos] = (dl[sel] - (dl[sel] // w) * w).astype(np.float32)
        nm[pos] = nmv[sel]
        ix[pos] = idx_vals[sel]

        idx_cols = []
        for gg in range(g_n):
            for hh in range(2):
                lo = int(call_base[gg, hh]) * 128
                ln = int(l_gh[gg, hh])
                if ln:
                    idx_cols.append(_wrap_idx(ix[lo:lo + ln]))
        idxw = (np.concatenate(idx_cols, axis=1) if idx_cols
                else np.zeros((128, 1), np.int16))
        per_core.append(dict(
            dc=np.ascontiguousarray(
                dc.reshape(t_tot, 128).T.astype(NPBF16)),
            nm=np.ascontiguousarray(
                nm.reshape(t_tot, 128).T.astype(NPBF16)),
            idx=idxw,
        ))
    return dict(nb=nb, g_n=g_n, t_cell=t_cell, t_tot=t_tot,
                tile_base=tile_base, t_call=t_call, l_gh=l_gh,
                call_base=call_base, off16=off16, it16=max(it16, 16),
                per_core=per_core)


def _build(plan1, plan2):
    ntmax = int(max(plan1["t_call"].max(), plan2["t_call"].max()))
    nc = bacc.Bacc("TRN2", target_bir_lowering=False, debug=False,
                   num_devices=CORES, num_swdge_queues=4)

    x_d = nc.dram_tensor("x", [N, D_IN], BF16, kind="ExternalInput").ap()
    w1_d = nc.dram_tensor("w1", [D_IN, 2 * D_HID], BF16, kind="ExternalInput").ap()
    w2a_d = nc.dram_tensor("w2a", [D_HID, D_HID], BF16, kind="ExternalInput").ap()
    w2b_d = nc.dram_tensor("w2b", [D_HID, D_HID], BF16, kind="ExternalInput").ap()
    wfc_d = nc.dram_tensor("wfc", [D_HID, D_OUT], BF16, kind="ExternalInput").ap()
    b1a_d = nc.dram_tensor("b1a", [D_HID, 1], F32, kind="ExternalInput").ap()
    b1b_d = nc.dram_tensor("b1b", [D_HID, 1], F32, kind="ExternalInput").ap()
    b2_d = nc.dram_tensor("b2", [D_HID, 1], F32, kind="ExternalInput").ap()
    bfc_d = nc.dram_tensor("bfc", [D_OUT, 1], F32, kind="ExternalInput").ap()
    id_d = nc.dram_tensor("ident", [128, 128], BF16, kind="ExternalInput").ap()
    isq_d = nc.dram_tensor("isqb", [128, CHUNK], BF16, kind="ExternalInput").ap()
    isqc_d = nc.dram_tensor("isqc", [128, NB], F32, kind="ExternalInput").ap()
    xloc_d = nc.dram_tensor("xloc", [CHUNK, D_IN], BF16, kind="ExternalInput").ap()
    idx1_d = nc.dram_tensor("idx1", [128, plan1["it16"]], I16, kind="ExternalInput").ap()
    idx2_d = nc.dram_tensor("idx2", [128, plan2["it16"]], I16, kind="ExternalInput").ap()
    dc1_d = nc.dram_tensor("dc1", [128, plan1["t_tot"]], BF16, kind="ExternalInput").ap()
    nm1_d = nc.dram_tensor("nm1", [128, plan1["t_tot"]], BF16, kind="ExternalInput").ap()
    dc2_d = nc.dram_tensor("dc2", [128, plan2["t_tot"]], BF16, kind="ExternalInput").ap()
    y_d = nc.dram_tensor("y", [D_OUT, CHUNK], F32, kind="ExternalOutput").ap()

    # halo-exchange internal DRAM (Shared address space for fast AllGather)
    t2loc = nc.dram_tensor("t2loc", [CHUNK, D_HID], BF16, kind="Internal").ap()
    agA = nc.dram_tensor("agA", [CORES * JSPLIT, D_HID], BF16,
                         kind="Internal", addr_space="Shared").ap()
    agB = nc.dram_tensor("agB", [CORES * (CHUNK - JSPLIT), D_HID], BF16,
                         kind="Internal", addr_space="Shared").ap()

    with tile.TileContext(nc) as tc:
        with (
            tc.tile_pool(name="const", bufs=1) as constp,
            tc.tile_pool(name="meta", bufs=1) as metap,
            tc.tile_pool(name="msgs", bufs=8) as msgsp,
            tc.tile_pool(name="ind", bufs=4) as indp,
            tc.tile_pool(name="eqt", bufs=2) as eqtp,
            tc.tile_pool(name="big", bufs=1) as bigp,
            tc.tile_pool(name="seg", bufs=2) as segp,
            tc.tile_pool(name="sm", bufs=3) as smp,
            tc.tile_pool(name="ps", bufs=4, space="PSUM") as psp,
            tc.tile_pool(name="pst", bufs=2, space="PSUM") as pstp,
        ):
            seg_len = [min(BPG * W, CHUNK - i * BPG * W) for i in range(NSEG)]
            seg_off = [BPG * W * i for i in range(NSEG)]

            # ---- constants ----
            ident = constp.tile([128, 128], BF16, tag="ident")
            nc.sync.dma_start(ident[:], id_d[:])
            w1_sb = constp.tile([D_IN, 2 * D_HID], BF16, tag="w1")
            nc.sync.dma_start(w1_sb[:], w1_d[:])
            w2a_sb = constp.tile([D_HID, D_HID], BF16, tag="w2a")
            nc.sync.dma_start(w2a_sb[:], w2a_d[:])
            w2b_sb = constp.tile([D_HID, D_HID], BF16, tag="w2b")
            nc.sync.dma_start(w2b_sb[:], w2b_d[:])
            wfc_sb = constp.tile([D_HID, D_OUT], BF16, tag="wfc")
            nc.sync.dma_start(wfc_sb[:], wfc_d[:])
            b1a_sb = constp.tile([D_HID, 1], F32, tag="b1a")
            nc.sync.dma_start(b1a_sb[:], b1a_d[:])
            b1b_sb = constp.tile([D_HID, 1], F32, tag="b1b")
            nc.sync.dma_start(b1b_sb[:], b1b_d[:])
            b2_sb = constp.tile([D_HID, 1], F32, tag="b2")
            nc.sync.dma_start(b2_sb[:], b2_d[:])
            bfc_sb = constp.tile([D_OUT, 1], F32, tag="bfc")
            nc.sync.dma_start(bfc_sb[:], bfc_d[:])
            isq_sb = constp.tile([128, CHUNK], BF16, tag="isq")
            nc.sync.dma_start(isq_sb[:], isq_d[:])

            iota_sb = constp.tile([128, ntmax * 128], BF16, tag="iota")
            nc.gpsimd.iota(iota_sb[:], [[0, ntmax], [1, 128]],
                           channel_multiplier=0,
                           allow_small_or_imprecise_dtypes=True)

            t2fm = bigp.tile([D_HID, CHUNK], BF16, tag="t2fm")

            # xs_fm[:, d] = isq_d * x_d  (feature-major local chunk, for the
            # conv1 self-loop contribution): load node-major 128-row tiles,
            # scale by isq (per-partition), PE-transpose to feature-major.
            isqc_sb = constp.tile([128, NB], F32, tag="isqc")
            nc.sync.dma_start(isqc_sb[:], isqc_d[:])
            xs_fm = bigp.tile([D_IN, CHUNK], BF16, tag="xsfm")
            for b in range(NB):
                wb = min(W, CHUNK - b * W)
                xn = smp.tile([128, D_IN], BF16, tag="xn")
                nc.sync.dma_start(xn[:wb, :], xloc_d[b * W:b * W + wb, :])
                xsn = smp.tile([128, D_IN], BF16, tag="xsn")
                nc.vector.tensor_scalar(xsn[:wb, :], xn[:wb, :],
                                        isqc_sb[:wb, b:b + 1], None,
                                        op0=mybir.AluOpType.mult)
                px = pstp.tile([128, 128], BF16, tag="ptr")
                nc.tensor.transpose(px[:, :wb], xsn[:wb, :], ident[:wb, :wb])
                nc.vector.tensor_copy(xs_fm[:, b * W:b * W + wb], px[:, :wb])

            qcount = [0]

            def _bc(ap_, col0, nt):
                """[128, T] tile -> [128, nt, 128] stride-0 broadcast view."""
                return bass.AP(ap_.tensor, ap_.offset + col0,
                               [list(ap_.ap[0]), [1, nt], [0, 128]])

            def _iota_v(nt):
                a = iota_sb[:]
                return bass.AP(a.tensor, a.offset,
                               [list(a.ap[0]), [128, nt], [1, 128]])

            def propagation(plan, idx_sb, dc_sb, nm_sb, src_views,
                            post_block, post_seg):
                g_n = plan["g_n"]
                t_cell, tile_base = plan["t_cell"], plan["tile_base"]
                l_gh, call_base, off16 = (plan["l_gh"], plan["call_base"],
                                          plan["off16"])
                for g in range(g_n):
                    msgs = {}
                    inds = {}
                    for h in (0, 1):
                        ln = int(l_gh[g, h])
                        if ln == 0:
                            continue
                        nt = ln // 128
                        cb = int(call_base[g, h])
                        mt = msgsp.tile([128, ntmax, D_IN], BF16, tag="msgs")
                        nc.gpsimd.dma_gather(
                            mt[:, :nt, :], src_views[h],
                            idx_sb[:, int(off16[g, h]):
                                   int(off16[g, h]) + ln // 16],
                            ln, ln, D_IN, elem_step=D_IN,
                            single_packet=False, queue_num=qcount[0] % 4,
                        )
                        qcount[0] += 1
                        msgs[h] = mt
                        it = indp.tile([128, ntmax, W], BF16, tag="ind")
                        if nm_sb is not None:
                            eq = eqtp.tile([128, ntmax, W], BF16, tag="eq")
                            nc.vector.tensor_tensor(
                                eq[:, :nt, :], _iota_v(nt), _bc(dc_sb[:], cb, nt),
                                op=mybir.AluOpType.is_equal)
                            nc.vector.tensor_tensor(
                                it[:, :nt, :], eq[:, :nt, :],
                                _bc(nm_sb[:], cb, nt),
                                op=mybir.AluOpType.mult)
                        else:
                            nc.vector.tensor_tensor(
                                it[:, :nt, :], _iota_v(nt), _bc(dc_sb[:], cb, nt),
                                op=mybir.AluOpType.is_equal)
                        inds[h] = it
                    for bl in range(BPG):
                        b = g * BPG + bl
                        if b >= NB:
                            break
                        wb = min(W, CHUNK - b * W)
                        n_t = int(t_cell[g, 0, bl] + t_cell[g, 1, bl])
                        if n_t == 0:
                            continue
                        ps = psp.tile([128, W], F32, tag="ps")
                        k = 0
                        for h in (0, 1):
                            tb = int(tile_base[g, h, bl])
                            cb = int(call_base[g, h])
                            for t in range(int(t_cell[g, h, bl])):
                                tl = tb - cb + t
                                nc.tensor.matmul(
                                    ps[:], msgs[h][:, tl, :],
                                    inds[h][:, tl, :],
                                    start=(k == 0), stop=(k == n_t - 1),
                                )
                                k += 1
                        post_block(g, b, wb, ps)
                    post_seg(g)

            # ================= conv1 =================
            agg1s = [bigp.tile([D_IN, seg_len[i]], BF16, tag=f"agg{i}",
                               name=f"agg{i}") for i in range(NSEG)]

            def c1_post_block(g, b, wb, ps):
                co = (b - g * BPG) * W
                u = smp.tile([D_IN, W], BF16, tag="u")
                nc.vector.tensor_tensor(u[:, :wb], ps[:, :wb],
                                        xs_fm[:, b * W:b * W + wb],
                                        op=mybir.AluOpType.add)
                nc.vector.tensor_tensor(
                    agg1s[g][:, co:co + wb], u[:, :wb],
                    isq_sb[:, b * W:b * W + wb], op=mybir.AluOpType.mult)

            def c1_post_seg(sg):
                ln = seg_len[sg]
                off = seg_off[sg]
                h1a = segp.tile([D_HID, BPG * W], BF16, tag="h1a")
                h1b = segp.tile([D_HID, BPG * W], BF16, tag="h1b")
                for s0 in range(0, ln, SLAB):
                    sl = min(SLAB, ln - s0)
                    pa = pstp.tile([128, SLAB], F32, tag="pst")
                    nc.tensor.matmul(pa[:, :sl], w1_sb[:, 0:D_HID],
                                     agg1s[sg][:, s0:s0 + sl])
                    nc.scalar.activation(h1a[:, s0:s0 + sl], pa[:, :sl],
                                         mybir.ActivationFunctionType.Relu,
                                         bias=b1a_sb[:, 0:1])
                    pb = pstp.tile([128, SLAB], F32, tag="pst")
                    nc.tensor.matmul(pb[:, :sl], w1_sb[:, D_HID:2 * D_HID],
                                     agg1s[sg][:, s0:s0 + sl])
                    nc.scalar.activation(h1b[:, s0:s0 + sl], pb[:, :sl],
                                         mybir.ActivationFunctionType.Relu,
                                         bias=b1b_sb[:, 0:1])
                # scale by isq (conv2 source prefold), then W2
                h1sa = segp.tile([D_HID, BPG * W], BF16, tag="h1sa")
                h1sb = segp.tile([D_HID, BPG * W], BF16, tag="h1sb")
                nc.vector.tensor_tensor(h1sa[:, :ln], h1a[:, :ln],
                                        isq_sb[:, off:off + ln],
                                        op=mybir.AluOpType.mult)
                nc.vector.tensor_tensor(h1sb[:, :ln], h1b[:, :ln],
                                        isq_sb[:, off:off + ln],
                                        op=mybir.AluOpType.mult)
                for s0 in range(0, ln, SLAB):
                    sl = min(SLAB, ln - s0)
                    pc = pstp.tile([128, SLAB], F32, tag="pst")
                    nc.tensor.matmul(pc[:, :sl], w2a_sb[:],
                                     h1sa[:, s0:s0 + sl],
                                     start=True, stop=False)
                    nc.tensor.matmul(pc[:, :sl], w2b_sb[:],
                                     h1sb[:, s0:s0 + sl],
                                     start=False, stop=True)
                    nc.scalar.activation(t2fm[:, off + s0:off + s0 + sl],
                                         pc[:, :sl],
                                         mybir.ActivationFunctionType.Copy)
                # transpose to node-major and stage to local DRAM
                for c0 in range(0, ln, 128):
                    cl = min(128, ln - c0)
                    pt = pstp.tile([128, 128], BF16, tag="ptr")
                    nc.tensor.transpose(pt[:cl, :],
                                        t2fm[:, off + c0:off + c0 + cl],
                                        ident[:])
                    tn = smp.tile([128, 128], BF16, tag="tn")
                    nc.vector.tensor_copy(tn[:cl, :], pt[:cl, :])
                    nc.sync.dma_start(t2loc[off + c0:off + c0 + cl, :],
                                      tn[:cl, :])
                if sg == AG_SEG_A:
                    nc.gpsimd.collective_compute(
                        "AllGather", mybir.AluOpType.bypass,
                        replica_groups=[list(range(CORES))],
                        ins=[t2loc[0:JSPLIT, :]], outs=[agA[:, :]])
                if sg == NSEG - 1:
                    nc.gpsimd.collective_compute(
                        "AllGather", mybir.AluOpType.bypass,
                        replica_groups=[list(range(CORES))],
                        ins=[t2loc[JSPLIT:CHUNK, :]], outs=[agB[:, :]])

            idx1_sb = metap.tile([128, plan1["it16"]], I16, tag="idx1")
            nc.sync.dma_start(idx1_sb[:], idx1_d[:])
            dc1_sb = metap.tile([128, plan1["t_tot"]], BF16, tag="dc1")
            nc.sync.dma_start(dc1_sb[:], dc1_d[:])
            nm1_sb = metap.tile([128, plan1["t_tot"]], BF16, tag="nm1")
            nc.sync.dma_start(nm1_sb[:], nm1_d[:])
            idx2_sb = metap.tile([128, plan2["it16"]], I16, tag="idx2")
            nc.sync.dma_start(idx2_sb[:], idx2_d[:])
            dc2_sb = metap.tile([128, plan2["t_tot"]], BF16, tag="dc2")
            nc.sync.dma_start(dc2_sb[:], dc2_d[:])

            propagation(plan1, idx1_sb, dc1_sb, nm1_sb,
                        [x_d[0:H1TAB, :], x_d[H1TAB:N, :]],
                        c1_post_block, c1_post_seg)

            # ================= conv2 =================
            out2 = [bigp.tile([D_HID, seg_len[i]], BF16, tag=f"agg{i}",
                              name=f"out2{i}") for i in range(NSEG)]

            def c2_post_block(g, b, wb, ps):
                co = (b - g * BPG) * W
                u = smp.tile([D_HID, W], BF16, tag="u")
                nc.vector.tensor_tensor(u[:, :wb], ps[:, :wb],
                                        t2fm[:, b * W:b * W + wb],
                                        op=mybir.AluOpType.add)
                v = smp.tile([D_HID, W], BF16, tag="v")
                nc.vector.tensor_tensor(v[:, :wb], u[:, :wb],
                                        isq_sb[:, b * W:b * W + wb],
                                        op=mybir.AluOpType.mult)
                nc.scalar.activation(out2[g][:, co:co + wb], v[:, :wb],
                                     mybir.ActivationFunctionType.Relu,
                                     bias=b2_sb[:, 0:1])

            def c2_post_seg(sg):
                ln = seg_len[sg]
                off = seg_off[sg]
                for s0 in range(0, ln, SLAB):
                    sl = min(SLAB, ln - s0)
                    pf = pstp.tile([D_OUT, SLAB], F32, tag="ptr")
                    nc.tensor.matmul(pf[:, :sl], wfc_sb[:],
                                     out2[sg][:, s0:s0 + sl])
                    yt = smp.tile([D_OUT, SLAB], F32, tag="yt")
                    nc.vector.tensor_scalar(yt[:, :sl], pf[:, :sl],
                                            bfc_sb[:, 0:1], None,
                                            op0=mybir.AluOpType.add)
                    nc.sync.dma_start(y_d[:, off + s0:off + s0 + sl],
                                      yt[:, :sl])

            propagation(plan2, idx2_sb, dc2_sb, None,
                        [agA[:, :], agB[:, :]],
                        c2_post_block, c2_post_seg)

    nc.compile()
    return nc


def _preprocess(x, W1, b1, W2, b2, Wfc, bfc, edge_index):
    src_e = edge_index[0]
    dst_e = edge_index[1]
    loops = np.arange(N, dtype=np.int64)
    dst_all = np.concatenate([dst_e, loops])
    deg = np.bincount(dst_all, minlength=N).astype(np.float32)
    isq = deg ** -0.5

    # conv1: edges only (self-loops folded into psum post-processing);
    # gather table = x rows split at 25000
    h1 = (src_e >= H1TAB).astype(np.int64)
    idx1 = src_e - h1 * H1TAB
    plan1 = _plan_conv(src_e, dst_e, isq[src_e], h1, idx1,
                       CORES, CHUNK, W, BPG)

    # conv2: edges only (self-loops folded into psum post-processing);
    # gather tables = AllGathered t2' split at local node JSPLIT
    core2 = src_e // CHUNK
    j2 = src_e % CHUNK
    h2 = (j2 >= JSPLIT).astype(np.int64)
    idx2 = np.where(h2 == 0, core2 * JSPLIT + j2,
                    core2 * (CHUNK - JSPLIT) + (j2 - JSPLIT))
    plan2 = _plan_conv(src_e, dst_e, isq[src_e], h2, idx2,
                       CORES, CHUNK, W, BPG)

    common = dict(
        x=np.ascontiguousarray(x.astype(NPBF16)),
        w1=np.ascontiguousarray(W1.astype(NPBF16)),
        w2a=np.ascontiguousarray(W2[:D_HID].astype(NPBF16)),
        w2b=np.ascontiguousarray(W2[D_HID:].astype(NPBF16)),
        wfc=np.ascontiguousarray(Wfc.astype(NPBF16)),
        b1a=np.ascontiguousarray(b1[:D_HID].reshape(D_HID, 1).astype(np.float32)),
        b1b=np.ascontiguousarray(b1[D_HID:].reshape(D_HID, 1).astype(np.float32)),
        b2=np.ascontiguousarray(b2.reshape(D_HID, 1).astype(np.float32)),
        bfc=np.ascontiguousarray(bfc.reshape(D_OUT, 1).astype(np.float32)),
        ident=np.eye(128, dtype=np.float32).astype(NPBF16),
    )
    xbf = common["x"]
    in_maps = []
    for m in range(CORES):
        im = dict(common)
        im["isqb"] = np.ascontiguousarray(
            np.broadcast_to(isq[m * CHUNK:(m + 1) * CHUNK][None, :],
                            (128, CHUNK)).astype(NPBF16))
        isql = isq[m * CHUNK:(m + 1) * CHUNK]
        isqc = np.zeros((128, NB), np.float32)
        for b in range(NB):
            wb = min(W, CHUNK - b * W)
            isqc[:wb, b] = isql[b * W:b * W + wb]
        im["isqc"] = isqc
        im["xloc"] = np.ascontiguousarray(xbf[m * CHUNK:(m + 1) * CHUNK])
        im["idx1"] = plan1["per_core"][m]["idx"]
        im["dc1"] = plan1["per_core"][m]["dc"]
        im["nm1"] = plan1["per_core"][m]["nm"]
        im["idx2"] = plan2["per_core"][m]["idx"]
        im["dc2"] = plan2["per_core"][m]["dc"]
        in_maps.append(im)
    return plan1, plan2, in_maps


_CACHE = {}


def kernel(x, W1, b1, W2, b2, Wfc, bfc, edge_index, trace=False, **_kw):
    x = np.asarray(x, dtype=np.float32)
    edge_index = np.asarray(edge_index).astype(np.int64)
    plan1, plan2, in_maps = _preprocess(
        x, np.asarray(W1), np.asarray(b1), np.asarray(W2), np.asarray(b2),
        np.asarray(Wfc), np.asarray(bfc), edge_index)
    key = ("nc", plan1["t_tot"], plan2["t_tot"])
    if key not in _CACHE:
        _CACHE[key] = _build(plan1, plan2)
    nc = _CACHE[key]
    res = run_bass_kernel_spmd(nc, in_maps, list(range(CORES)), trace=trace)
    y = np.concatenate([res.results[m]["y"].T for m in range(CORES)], axis=0)
    if trace:
        kernel.last_exec_time_ns = res.exec_time_ns
        kernel.last_results = res
    return y.astype(np.float32)
